# revision 26
# baseline (speedup 1.0000x reference)
"""ALoraLinear on 8 TRN2 NeuronCores.

y = x @ W^T + b + mask ⊙ ((x @ A^T) @ B_w^T) * 2.0
  B=4, S=4096, D_IN=D_OUT=4096, R=32; mask = per-sample tail of the sequence.

Strategy: pure data-parallel over the 16384 flattened tokens (2048/core), no
collectives. Host-side prep (free w.r.t. HW time): transpose x and W into
partition-tiled [128, K/128, free] bf16 layouts; fold the bias into the LoRA
matmul (B_w^T with the bias as row 32, zero rows 33..127 memset on-device,
matched by a constant-1 row 32 in the masked LoRA activations); fold mask*2.0
into a per-token vector applied to the tiny LoRA activation u^T = A @ x^T.

Per core: for each of 8x16 output tiles [128 tok, 512 dout], accumulate 32
K-tile matmuls of x^T·W^T plus one LoRA matmul into the same PSUM bank,
copy to SBUF on the vector engine, DMA out. Compute-bound at bf16
(~68.7 GFLOP/core vs 78.6 TFLOP/s peak).

Engine split: sync issues wt prefetch, gpsimd issues the x^T stream, scalar
issues output DMAs, vector evicts PSUM; W^T streams as 4-k-tile chunk DMAs
so block turnover costs 8 issues instead of 32.
"""

import numpy as np
import ml_dtypes

N_CORES = 8
B, S, D_IN, D_OUT, R = 4, 4096, 4096, 4096, 32
SCALING = 2.0
P = 128
TOKC = (B * S) // N_CORES  # 2048 tokens per core
KT = D_IN // P  # 32 k-tiles
KW = 4  # k-tiles per wt chunk DMA
NKW = KT // KW  # 8 chunks per n-block
NB = D_OUT // 512  # 8 n-blocks of 512
MT = TOKC // P  # 16 m-tiles of 128 tokens
NCHUNK = TOKC // 512  # 4 chunks for the LoRA activation

_COMPILED = None


def _build():
    import concourse.bacc as bacc
    import concourse.mybir as mybir
    import concourse.tile as tile

    bf16 = mybir.dt.bfloat16
    f32 = mybir.dt.float32

    nc = bacc.Bacc("TRN2", target_bir_lowering=False, debug=False)

    xt_d = nc.dram_tensor("xt", [P, KT, TOKC], bf16, kind="ExternalInput")
    wt_d = nc.dram_tensor("wt", [P, KT, D_OUT], bf16, kind="ExternalInput")
    at_d = nc.dram_tensor("at", [P, KT, R], bf16, kind="ExternalInput")
    bwt_d = nc.dram_tensor("bwt", [R + 1, D_OUT], bf16, kind="ExternalInput")
    mask_d = nc.dram_tensor("mask", [P, TOKC], bf16, kind="ExternalInput")
    out_d = nc.dram_tensor("out", [TOKC, D_OUT], f32, kind="ExternalOutput")

    with tile.TileContext(nc) as tc:
        with (
            tc.tile_pool(name="const", bufs=1) as const,
            tc.tile_pool(name="xtp", bufs=1) as xtp,
            tc.tile_pool(name="utp", bufs=1) as utp,
            tc.tile_pool(name="wtp", bufs=NKW + 2) as wtp,
            tc.tile_pool(name="outp", bufs=3) as outp,
            tc.tile_pool(name="psum", bufs=8, space="PSUM") as psum,
        ):
            at_sb = const.tile([P, KT, R], bf16, name="at_sb")
            bwt_sb = const.tile([P, D_OUT], bf16, name="bwt_sb")
            mask_sb = const.tile([P, TOKC], bf16, name="mask_sb")
            xt_sb = xtp.tile([P, KT, TOKC], bf16, name="xt_sb")
            ut_sb = utp.tile([P, TOKC], bf16, name="ut_sb")

            def load_wt_chunk(n, c):
                wt = wtp.tile([P, KW, 512], bf16, name="wt_sb")
                nc.sync.dma_start(
                    wt[:],
                    wt_d.ap()[:, c * KW : (c + 1) * KW, n * 512 : (n + 1) * 512],
                )
                return wt

            def emit_group_mm(ps, m, k, wt_chunks):
                nc.tensor.matmul(
                    ps[:],
                    xt_sb[:, k, m * P : (m + 1) * P],
                    wt_chunks[k // KW][:, k % KW, :],
                    start=(k == 0),
                    stop=False,
                )

            def emit_group_tail(ps, n, m):
                nsl = slice(n * 512, (n + 1) * 512)
                msl = slice(m * P, (m + 1) * P)
                nc.tensor.matmul(
                    ps[:], ut_sb[:, msl], bwt_sb[:, nsl], start=False, stop=True
                )
                ot = outp.tile([P, 512], f32, name="ot")
                nc.vector.tensor_copy(ot[:], ps[:])
                # scalar engine issues output DMAs so their sem-waits never
                # stall the sync engine's in-order wt-prefetch stream
                nc.scalar.dma_start(out_d.ap()[msl, nsl], ot[:])

            # PE clock warmup: the HAM gate holds the PE at half clock until
            # ~3.4us of sustained activity. The first ~10us are DMA-only, so
            # run a dense burst of throwaway matmuls (no DMA deps) to reach
            # full clock before the real ramp matmuls arrive.
            warm_sb = const.tile([P, 512], bf16, name="warm_sb")
            nc.vector.memset(warm_sb[:], 0.0)
            wps = psum.tile([P, 512], f32, name="ps")
            for i in range(20):
                nc.tensor.matmul(
                    wps[:], warm_sb[:, 0:P], warm_sb[:], start=(i == 0), stop=(i == 19)
                )

            # LoRA operands use K=128 (Fast Weight Load needs a full 128-row
            # stationary) but only rows 0..31 (ranks) and 32 (bias) are real.
            # Zero rows 33..127 once on-device so no NaN*0 can leak, then
            # overlay: bwt rows 0..32 from its 33-row DRAM tensor, ut row 32
            # := 1.0 via DMA of the host ones row (compute engines can't
            # address partition ranges starting mid-strip).
            for p0 in (32, 64, 96):
                nc.vector.memset(ut_sb[p0 : p0 + 32, :], 0.0)
                nc.vector.memset(bwt_sb[p0 : p0 + 32, :], 0.0)
            nc.sync.dma_start(ut_sb[32:33, :], mask_d.ap()[127:128, :])

            # ---- Ramp phase: n-block 0, overlapped with the x^T DMA stream.
            # Head DMAs are emitted in PE need-order, interleaved per k
            # (at[k], wt0 chunk, xt[k]), so the in-order PE can run 4
            # LoRA-activation matmuls (u^T = A_pad @ x^T) plus the k-matmuls
            # of main groups m=0..3 (8 PSUM banks total) chasing the DMA
            # stream instead of idling until x^T is resident.
            wt_chunks0 = []
            for k in range(KT):
                nc.sync.dma_start(at_sb[:, k : k + 1, :], at_d.ap()[:, k : k + 1, :])
                if k % KW == 0:
                    wt_chunks0.append(load_wt_chunk(0, k // KW))
                # alternate the x^T issue stream between gpsimd and scalar
                # (idle until the first output DMA at ~65us) so per-dma_start
                # issue latency doesn't serialize the ramp
                xeng = nc.gpsimd if k % 2 == 0 else nc.scalar
                if k < 2:
                    # quarter-split the first k-tiles so the first ramp
                    # matmuls (which read 512-token subtiles) fire early
                    for q in range(4):
                        qsl = slice(q * 512, (q + 1) * 512)
                        xeng.dma_start(
                            xt_sb[:, k : k + 1, qsl], xt_d.ap()[:, k : k + 1, qsl]
                        )
                else:
                    xeng.dma_start(
                        xt_sb[:, k : k + 1, :], xt_d.ap()[:, k : k + 1, :]
                    )
            nc.sync.dma_start(bwt_sb[0 : R + 1, :], bwt_d.ap()[:])
            nc.sync.dma_start(mask_sb[:], mask_d.ap()[:])

            RAMP_M = 4
            ups = [psum.tile([P, 512], f32, name="ps") for _ in range(NCHUNK)]
            mps0 = [psum.tile([P, 512], f32, name="ps") for _ in range(RAMP_M)]
            for k in range(KT):
                for c in range(NCHUNK):
                    nc.tensor.matmul(
                        ups[c][0:R, :],
                        at_sb[:, k, :],
                        xt_sb[:, k, c * 512 : (c + 1) * 512],
                        start=(k == 0),
                        stop=(k == KT - 1),
                    )
                for m in range(RAMP_M):
                    emit_group_mm(mps0[m], m, k, wt_chunks0)

            # masked+scaled LoRA activation, bf16 (real rows 0..31 only).
            for c in range(NCHUNK):
                sl = slice(c * 512, (c + 1) * 512)
                nc.vector.tensor_mul(ut_sb[0:32, sl], ups[c][0:32, :], mask_sb[0:32, sl])

            for m in range(RAMP_M):
                emit_group_tail(mps0[m], 0, m)

            # ---- Steady state: remaining groups of n=0, then n=1..7.
            for m in range(RAMP_M, MT):
                ps = psum.tile([P, 512], f32, name="ps")
                for k in range(KT):
                    emit_group_mm(ps, m, k, wt_chunks0)
                emit_group_tail(ps, 0, m)

            for n in range(1, NB):
                wt_chunks = [load_wt_chunk(n, c) for c in range(NKW)]
                for m in range(MT):
                    ps = psum.tile([P, 512], f32, name="ps")
                    for k in range(KT):
                        emit_group_mm(ps, m, k, wt_chunks)
                    emit_group_tail(ps, n, m)

    nc.compile()
    return nc


def _get_compiled():
    global _COMPILED
    if _COMPILED is None:
        _COMPILED = _build()
    return _COMPILED


def _tile_kx(a_t: np.ndarray) -> np.ndarray:
    """[K, F] -> partition-tiled [128, K/128, F] bf16, C-contiguous."""
    k, f = a_t.shape
    return np.ascontiguousarray(
        a_t.reshape(k // P, P, f).transpose(1, 0, 2)
    ).astype(ml_dtypes.bfloat16)


def _prepare_in_maps(x, alora_offsets, W, b, A, B_w):
    bf = ml_dtypes.bfloat16
    xf = np.asarray(x, dtype=np.float32).reshape(B * S, D_IN)

    wt_np = _tile_kx(np.asarray(W, dtype=np.float32).T)  # [128, 32, 4096]

    at_np = _tile_kx(np.asarray(A, dtype=np.float32).T)  # [128, 32, 32]

    bwt_np = np.zeros((R + 1, D_OUT), dtype=np.float32)
    bwt_np[:R] = np.asarray(B_w, dtype=np.float32).T
    bwt_np[R] = np.asarray(b, dtype=np.float32)  # bias row (partition 32)
    bwt_np = bwt_np.astype(bf)

    # per-token mask * SCALING over the flattened (b, s) axis
    offs = np.asarray(alora_offsets, dtype=np.int64)
    kk = np.minimum(offs, S)
    pos = np.arange(S, dtype=np.int64)
    mask_full = (pos[None, :] >= (S - kk)[:, None]).astype(np.float32) * SCALING
    mask_full = mask_full.reshape(B * S)

    in_maps = []
    for c in range(N_CORES):
        tok = slice(c * TOKC, (c + 1) * TOKC)
        xt_np = _tile_kx(xf[tok].T)  # [128, 32, 2048]
        mask_np = np.broadcast_to(mask_full[tok], (P, TOKC)).copy()
        mask_np[P - 1] = 1.0  # ones row, DMA'd into ut row 32 (bias path)
        mask_np = np.ascontiguousarray(mask_np).astype(bf)
        in_maps.append(
            {"xt": xt_np, "wt": wt_np, "at": at_np, "bwt": bwt_np, "mask": mask_np}
        )
    return in_maps


def _run(inputs: dict, trace: bool = False):
    from concourse.bass_utils import run_bass_kernel_spmd

    nc = _get_compiled()
    in_maps = _prepare_in_maps(**inputs)
    res = None
    for attempt in range(3):
        try:
            res = run_bass_kernel_spmd(
                nc, in_maps, core_ids=list(range(N_CORES)), trace=trace
            )
            break
        except Exception:
            # transient device faults (e.g. NRT_EXEC_UNIT_UNRECOVERABLE)
            # clear on retry; re-raise only if persistent
            if attempt == 2:
                raise
    out = np.concatenate(
        [res.results[c]["out"] for c in range(N_CORES)], axis=0
    ).reshape(B, S, D_OUT)
    return out, res


def kernel(x, alora_offsets, W, b, A, B_w) -> np.ndarray:
    out, _ = _run(
        {"x": x, "alora_offsets": alora_offsets, "W": W, "b": b, "A": A, "B_w": B_w}
    )
    return out


# revision 27
# speedup vs baseline: 1.0142x; 1.0142x over previous
"""ALoraLinear on 8 TRN2 NeuronCores.

y = x @ W^T + b + mask ⊙ ((x @ A^T) @ B_w^T) * 2.0
  B=4, S=4096, D_IN=D_OUT=4096, R=32; mask = per-sample tail of the sequence.

Strategy: pure data-parallel over the 16384 flattened tokens (2048/core), no
collectives. Host-side prep (free w.r.t. HW time): transpose x and W into
partition-tiled [128, K/128, free] bf16 layouts; fold the bias into the LoRA
matmul (B_w^T with the bias as row 32, zero rows 33..127 memset on-device,
matched by a constant-1 row 32 in the masked LoRA activations); fold mask*2.0
into a per-token vector applied to the tiny LoRA activation u^T = A @ x^T.

Per core: for each of 8x16 output tiles [128 tok, 512 dout], accumulate 32
K-tile matmuls of x^T·W^T plus one LoRA matmul into the same PSUM bank,
copy to SBUF on the vector engine, DMA out. Compute-bound at bf16
(~68.7 GFLOP/core vs 78.6 TFLOP/s peak).

Engine split: sync issues wt prefetch, gpsimd issues the x^T stream, scalar
issues output DMAs, vector evicts PSUM; W^T streams as 4-k-tile chunk DMAs
so block turnover costs 8 issues instead of 32.
"""

import numpy as np
import ml_dtypes

N_CORES = 8
B, S, D_IN, D_OUT, R = 4, 4096, 4096, 4096, 32
SCALING = 2.0
P = 128
TOKC = (B * S) // N_CORES  # 2048 tokens per core
KT = D_IN // P  # 32 k-tiles
KW = 4  # k-tiles per wt chunk DMA
NKW = KT // KW  # 8 chunks per n-block
NB = D_OUT // 512  # 8 n-blocks of 512
MT = TOKC // P  # 16 m-tiles of 128 tokens
NCHUNK = TOKC // 512  # 4 chunks for the LoRA activation

_COMPILED = None


def _build():
    import concourse.bacc as bacc
    import concourse.mybir as mybir
    import concourse.tile as tile

    bf16 = mybir.dt.bfloat16
    f32 = mybir.dt.float32

    nc = bacc.Bacc("TRN2", target_bir_lowering=False, debug=False)

    xt_d = nc.dram_tensor("xt", [P, KT, TOKC], bf16, kind="ExternalInput")
    wt_d = nc.dram_tensor("wt", [P, KT, D_OUT], bf16, kind="ExternalInput")
    at_d = nc.dram_tensor("at", [P, KT, R], bf16, kind="ExternalInput")
    bwt_d = nc.dram_tensor("bwt", [R + 1, D_OUT], bf16, kind="ExternalInput")
    mask_d = nc.dram_tensor("mask", [P, TOKC], bf16, kind="ExternalInput")
    out_d = nc.dram_tensor("out", [TOKC, D_OUT], f32, kind="ExternalOutput")

    with tile.TileContext(nc) as tc:
        with (
            tc.tile_pool(name="const", bufs=1) as const,
            tc.tile_pool(name="xtp", bufs=1) as xtp,
            tc.tile_pool(name="utp", bufs=1) as utp,
            tc.tile_pool(name="wtp", bufs=NKW + 2) as wtp,
            tc.tile_pool(name="outp", bufs=3) as outp,
            tc.tile_pool(name="psum", bufs=8, space="PSUM") as psum,
        ):
            at_sb = const.tile([P, KT, R], bf16, name="at_sb")
            bwt_sb = const.tile([P, D_OUT], bf16, name="bwt_sb")
            mask_sb = const.tile([P, TOKC], bf16, name="mask_sb")
            xt_sb = xtp.tile([P, KT, TOKC], bf16, name="xt_sb")
            ut_sb = utp.tile([P, TOKC], bf16, name="ut_sb")

            def load_wt_chunk(n, c):
                wt = wtp.tile([P, KW, 512], bf16, name="wt_sb")
                nc.sync.dma_start(
                    wt[:],
                    wt_d.ap()[:, c * KW : (c + 1) * KW, n * 512 : (n + 1) * 512],
                )
                return wt

            def emit_group_mm(ps, m, k, wt_chunks):
                nc.tensor.matmul(
                    ps[:],
                    xt_sb[:, k, m * P : (m + 1) * P],
                    wt_chunks[k // KW][:, k % KW, :],
                    start=(k == 0),
                    stop=False,
                )

            def emit_group_tail(ps, n, m):
                nsl = slice(n * 512, (n + 1) * 512)
                msl = slice(m * P, (m + 1) * P)
                nc.tensor.matmul(
                    ps[:], ut_sb[:, msl], bwt_sb[:, nsl], start=False, stop=True
                )
                ot = outp.tile([P, 512], f32, name="ot")
                nc.vector.tensor_copy(ot[:], ps[:])
                # scalar engine issues output DMAs so their sem-waits never
                # stall the sync engine's in-order wt-prefetch stream
                nc.scalar.dma_start(out_d.ap()[msl, nsl], ot[:])

            # PE clock warmup: the HAM gate holds the PE at half clock until
            # ~3.4us of sustained activity. The first ~10us are DMA-only, so
            # run a dense burst of throwaway matmuls (no DMA deps) to reach
            # full clock before the real ramp matmuls arrive.
            warm_sb = const.tile([P, 512], bf16, name="warm_sb")
            nc.vector.memset(warm_sb[:], 0.0)
            wps = psum.tile([P, 512], f32, name="ps")
            for i in range(20):
                nc.tensor.matmul(
                    wps[:], warm_sb[:, 0:P], warm_sb[:], start=(i == 0), stop=(i == 19)
                )

            # LoRA operands use K=128 (Fast Weight Load needs a full 128-row
            # stationary) but only rows 0..31 (ranks) and 32 (bias) are real.
            # Zero rows 33..127 once on-device so no NaN*0 can leak, then
            # overlay: bwt rows 0..32 from its 33-row DRAM tensor, ut row 32
            # := 1.0 via DMA of the host ones row (compute engines can't
            # address partition ranges starting mid-strip).
            for p0 in (32, 64, 96):
                nc.vector.memset(ut_sb[p0 : p0 + 32, :], 0.0)
                nc.vector.memset(bwt_sb[p0 : p0 + 32, :], 0.0)
            nc.sync.dma_start(ut_sb[32:33, :], mask_d.ap()[127:128, :])

            # ---- Ramp phase: n-block 0, overlapped with the x^T DMA stream.
            # Head DMAs are emitted in PE need-order, interleaved per k
            # (at[k], wt0 chunk, xt[k]), so the in-order PE can run 4
            # LoRA-activation matmuls (u^T = A_pad @ x^T) plus the k-matmuls
            # of main groups m=0..3 (8 PSUM banks total) chasing the DMA
            # stream instead of idling until x^T is resident.
            wt_chunks0 = []
            for k in range(KT):
                nc.sync.dma_start(at_sb[:, k : k + 1, :], at_d.ap()[:, k : k + 1, :])
                if k % KW == 0:
                    wt_chunks0.append(load_wt_chunk(0, k // KW))
                if k < 2:
                    # quarter-split the first k-tiles so the first ramp
                    # matmuls (which read 512-token subtiles) fire early
                    for q in range(4):
                        qsl = slice(q * 512, (q + 1) * 512)
                        nc.gpsimd.dma_start(
                            xt_sb[:, k : k + 1, qsl], xt_d.ap()[:, k : k + 1, qsl]
                        )
                else:
                    nc.gpsimd.dma_start(
                        xt_sb[:, k : k + 1, :], xt_d.ap()[:, k : k + 1, :]
                    )
            nc.sync.dma_start(bwt_sb[0 : R + 1, :], bwt_d.ap()[:])
            nc.sync.dma_start(mask_sb[:], mask_d.ap()[:])

            RAMP_M = 4
            ups = [psum.tile([P, 512], f32, name="ps") for _ in range(NCHUNK)]
            mps0 = [psum.tile([P, 512], f32, name="ps") for _ in range(RAMP_M)]
            for k in range(KT):
                for c in range(NCHUNK):
                    nc.tensor.matmul(
                        ups[c][0:R, :],
                        at_sb[:, k, :],
                        xt_sb[:, k, c * 512 : (c + 1) * 512],
                        start=(k == 0),
                        stop=(k == KT - 1),
                    )
                for m in range(RAMP_M):
                    emit_group_mm(mps0[m], m, k, wt_chunks0)

            # masked+scaled LoRA activation, bf16 (real rows 0..31 only).
            for c in range(NCHUNK):
                sl = slice(c * 512, (c + 1) * 512)
                nc.vector.tensor_mul(ut_sb[0:32, sl], ups[c][0:32, :], mask_sb[0:32, sl])

            for m in range(RAMP_M):
                emit_group_tail(mps0[m], 0, m)

            # ---- Steady state: remaining groups of n=0, then n=1..7.
            for m in range(RAMP_M, MT):
                ps = psum.tile([P, 512], f32, name="ps")
                for k in range(KT):
                    emit_group_mm(ps, m, k, wt_chunks0)
                emit_group_tail(ps, 0, m)

            for n in range(1, NB):
                wt_chunks = [load_wt_chunk(n, c) for c in range(NKW)]
                for m in range(MT):
                    ps = psum.tile([P, 512], f32, name="ps")
                    for k in range(KT):
                        emit_group_mm(ps, m, k, wt_chunks)
                    emit_group_tail(ps, n, m)

    nc.compile()
    return nc


def _get_compiled():
    global _COMPILED
    if _COMPILED is None:
        _COMPILED = _build()
    return _COMPILED


def _tile_kx(a_t: np.ndarray) -> np.ndarray:
    """[K, F] -> partition-tiled [128, K/128, F] bf16, C-contiguous."""
    k, f = a_t.shape
    return np.ascontiguousarray(
        a_t.reshape(k // P, P, f).transpose(1, 0, 2)
    ).astype(ml_dtypes.bfloat16)


def _prepare_in_maps(x, alora_offsets, W, b, A, B_w):
    bf = ml_dtypes.bfloat16
    xf = np.asarray(x, dtype=np.float32).reshape(B * S, D_IN)

    wt_np = _tile_kx(np.asarray(W, dtype=np.float32).T)  # [128, 32, 4096]

    at_np = _tile_kx(np.asarray(A, dtype=np.float32).T)  # [128, 32, 32]

    bwt_np = np.zeros((R + 1, D_OUT), dtype=np.float32)
    bwt_np[:R] = np.asarray(B_w, dtype=np.float32).T
    bwt_np[R] = np.asarray(b, dtype=np.float32)  # bias row (partition 32)
    bwt_np = bwt_np.astype(bf)

    # per-token mask * SCALING over the flattened (b, s) axis
    offs = np.asarray(alora_offsets, dtype=np.int64)
    kk = np.minimum(offs, S)
    pos = np.arange(S, dtype=np.int64)
    mask_full = (pos[None, :] >= (S - kk)[:, None]).astype(np.float32) * SCALING
    mask_full = mask_full.reshape(B * S)

    in_maps = []
    for c in range(N_CORES):
        tok = slice(c * TOKC, (c + 1) * TOKC)
        xt_np = _tile_kx(xf[tok].T)  # [128, 32, 2048]
        mask_np = np.broadcast_to(mask_full[tok], (P, TOKC)).copy()
        mask_np[P - 1] = 1.0  # ones row, DMA'd into ut row 32 (bias path)
        mask_np = np.ascontiguousarray(mask_np).astype(bf)
        in_maps.append(
            {"xt": xt_np, "wt": wt_np, "at": at_np, "bwt": bwt_np, "mask": mask_np}
        )
    return in_maps


def _run(inputs: dict, trace: bool = False):
    from concourse.bass_utils import run_bass_kernel_spmd

    nc = _get_compiled()
    in_maps = _prepare_in_maps(**inputs)
    res = None
    for attempt in range(3):
        try:
            res = run_bass_kernel_spmd(
                nc, in_maps, core_ids=list(range(N_CORES)), trace=trace
            )
            break
        except Exception:
            # transient device faults (e.g. NRT_EXEC_UNIT_UNRECOVERABLE)
            # clear on retry; re-raise only if persistent
            if attempt == 2:
                raise
    out = np.concatenate(
        [res.results[c]["out"] for c in range(N_CORES)], axis=0
    ).reshape(B, S, D_OUT)
    return out, res


def kernel(x, alora_offsets, W, b, A, B_w) -> np.ndarray:
    out, _ = _run(
        {"x": x, "alora_offsets": alora_offsets, "W": W, "b": b, "A": A, "B_w": B_w}
    )
    return out
